# revision 7
# baseline (speedup 1.0000x reference)
"""Pairwise Euclidean distance kernel for Trainium2 (8 NeuronCores, SPMD).

Computes out[i, j] = ||mapping[i] - mapping[j]|| for mapping [8192, 512] fp32.

Strategy ("raw-gram fp8 tournament"):

  - Round-robin tournament cover of the symmetric output: stripe s (512
    rows) owns the column blocks at cyclic distance 0..7 (plus distance 8
    for s < 8). Core c takes stripes {c, c+8}, giving EVERY core an
    identical 9-block + 8-block job structure once its input is rotated
    by 512*c columns. The unowned half is mirrored from the transpose on
    the host; nothing is computed twice.
  - The only device input is the rotated fp8(e4m3) point matrix
    T_c [128, 4, 8192] (4 MB). Both matmul operands are slices of it:
    stationary = own rows (local cols [0,512) / [4096,4608)), moving =
    local col range. fp8 DoubleRow matmuls contract 256 dims per
    instruction (2x bf16 throughput).
  - The device outputs the RAW GRAM g = a_i . a_j in fp8 — no norms, no
    relu, no sqrt on device. The epilogue is a single fp32->fp8 convert
    per psum supertile, column-split across DVE (clamping tensor_scalar)
    and ScalarE (Copy activation) so both engines run in parallel.
  - Host computes d = sqrt(max(sq_i + sq_j - 2 g, 0)) in fp32 from the
    fp8-rounded points' exact norms, overwrites the diagonal with exact
    0, recomputes any |g| >= 224 pair exactly (fp8 overflow guard; none
    occur off-diagonal for this data), and mirrors the lower triangle.
  - A post-compile pass drops back-to-back redundant LDWEIGHTS so runs of
    matmuls sharing one stationary operand pipeline on the PE array.
"""

import numpy as np
import ml_dtypes

N = 8192
D = 512
P = 128
KT = D // P                    # 4 k-subtiles of 128
NCORES = 8
SW = 512                       # stripe width (rows per stripe)
NSTRIPES = N // SW             # 16
MT = SW // P                   # 4 m-tiles per stripe
NSUB = 512                     # matmul free dim / psum bank (fp32)
TOTC = 17 * SW                 # 8704 output cols per core (9 + 8 blocks)

# (lhs local col, rhs local col, width, flat out col) per supertile.
# jobA: stripe c   -> local cols [0, 4608)   (9 blocks)
# jobB: stripe c+8 -> local cols [4096, 8192) (8 blocks)
SUPS = [
    (0,    0,    2048, 0),
    (0,    2048, 2048, 2048),
    (0,    4096, 512,  4096),
    (4096, 4096, 2048, 4608),
    (4096, 6144, 2048, 6656),
]

_compiled = None


def _dedup_ldweights(nc):
    """Remove back-to-back redundant weight loads (see baseline docstring)."""
    import concourse.mybir as mybir

    def sig(ldw):
        w = ldw.ins[0]
        return (w.memref, w.offset, str(w.ap), str(w.dtype),
                str(getattr(ldw, "perf_mode", None)),
                str(getattr(ldw, "is_transpose", None)),
                str(getattr(ldw, "tile_position", None)))

    removed = 0
    for f in nc.m.functions:
        for blk in f.blocks:
            last = None
            keep = []
            for inst in blk.instructions:
                if isinstance(inst, mybir.InstLdweights):
                    si = inst.sync_info
                    clean = si is None or (not si.on_wait and not si.on_update)
                    s = sig(inst)
                    if clean and last is not None and s == last:
                        removed += 1
                        continue
                    last = s
                elif isinstance(inst, mybir.InstMatmult):
                    if getattr(inst, "is_transpose", None):
                        last = None
                keep.append(inst)
            blk.instructions[:] = keep
    return removed


def _build():
    import concourse.mybir as mybir
    import concourse.tile as tile
    from concourse import bacc

    nc = bacc.Bacc()
    t_d = nc.dram_tensor("t", [P, KT, N], mybir.dt.float8e4,
                         kind="ExternalInput")
    out_d = nc.dram_tensor("out", [SW, TOTC], mybir.dt.float8e4,
                           kind="ExternalOutput")
    DR = mybir.MatmulPerfMode.DoubleRow

    with tile.TileContext(nc) as tc:
        with (
            tc.tile_pool(name="tbuf", bufs=1) as tp,
            tc.tile_pool(name="stage", bufs=4) as stagep,
            tc.tile_pool(name="psum", bufs=4, space="PSUM") as psump,
        ):
            t = tp.tile([P, KT, N], mybir.dt.float8e4, tag="t")
            # Input chunks ordered by first use; issued alternately on the
            # SP and Activation HWDGE queues so issue (~0.6us each) overlaps.
            for i, (k0, c0, c1) in enumerate([
                (0, 0, 2048), (2, 0, 2048),
                (0, 2048, 4608), (2, 2048, 4608),
                (0, 4608, N), (2, 4608, N),
            ]):
                eng = nc.sync if i % 2 == 0 else nc.scalar
                eng.dma_start(t[:, k0:k0 + 2, c0:c1], t_d[:, k0:k0 + 2, c0:c1])

            for lhs0, rhs0, w, oc0 in SUPS:
                for m in range(MT):
                    lm = lhs0 + m * P
                    # psum as independently-drained 2-bank halves -> finer
                    # recycling (bufs=4 covers two supertiles in flight).
                    nh = (w + 1023) // 1024
                    pss = []
                    for _ in range(nh):
                        psh = psump.tile([P, 1024], mybir.dt.float32, tag="ps")
                        pss.append(psh)
                    for kk in range(2):
                        ksl = slice(2 * kk, 2 * kk + 2)
                        for b0 in range(0, w, NSUB):
                            ps = pss[b0 // 1024]
                            pb = b0 % 1024
                            nc.tensor.matmul(
                                ps[:, pb:pb + NSUB],
                                t[:, ksl, lm:lm + P],
                                t[:, ksl, rhs0 + b0:rhs0 + b0 + NSUB],
                                start=(kk == 0),
                                stop=(kk == 1),
                                perf_mode=DR,
                            )
                    ob = stagep.tile([P, 2048], mybir.dt.float8e4, tag="ob")
                    # fp32->fp8 convert: DVE takes the first half (with a
                    # +-240 clamp), ScalarE the second (and all of the
                    # narrow supertile, where only one half exists).
                    if w == 2048:
                        nc.vector.tensor_scalar(
                            ob[:, 0:1024], pss[0][:], -240.0, 240.0,
                            mybir.AluOpType.max, mybir.AluOpType.min,
                        )
                        nc.scalar.activation(
                            ob[:, 1024:2048], pss[1][:],
                            mybir.ActivationFunctionType.Copy,
                        )
                    else:
                        nc.scalar.activation(
                            ob[:, 0:w], pss[0][:, 0:w],
                            mybir.ActivationFunctionType.Copy,
                        )
                    nc.sync.dma_start(
                        out_d[m * P:(m + 1) * P, oc0:oc0 + w], ob[:, 0:w])

    nc.compile()
    _dedup_ldweights(nc)
    return nc


def _prep_inputs(mapping):
    """Host-side shard/layout: rotated fp8 T per core."""
    f8 = ml_dtypes.float8_e4m3
    X8 = mapping.astype(f8)                                     # [N, D]
    in_maps = []
    idx = np.arange(N)
    for c in range(NCORES):
        Xr = X8[(idx + SW * c) % N]                             # [N, D] rotated
        # t[p, k, n] = point (512c+n)'s dim (128k + p)
        t = np.ascontiguousarray(Xr.T.reshape(KT, P, N).transpose(1, 0, 2))
        in_maps.append({"t": t})
    return in_maps


def kernel(mapping: np.ndarray) -> np.ndarray:
    from concourse.bass_utils import run_bass_kernel_spmd

    global _compiled
    mapping = np.asarray(mapping, dtype=np.float32)
    assert mapping.shape == (N, D)
    if _compiled is None:
        _compiled = _build()
    in_maps = _prep_inputs(mapping)
    res = run_bass_kernel_spmd(_compiled, in_maps, list(range(NCORES)))
    return assemble(mapping, res.results)


def assemble(mapping, results):
    """Host epilogue: scatter gram blocks, d = sqrt(max(sqi+sqj-2g, 0))."""
    f8 = ml_dtypes.float8_e4m3
    Xd = mapping.astype(f8).astype(np.float32)                  # fp8-rounded pts
    sq = np.einsum("nd,nd->n", Xd, Xd, dtype=np.float32)        # [N] exact norms

    G = np.full((N, N), np.nan, dtype=np.float32)
    owned = np.zeros((N, N), dtype=bool)
    for c in range(NCORES):
        blk = np.nan_to_num(
            results[c]["out"].astype(np.float32),
            nan=1e6, posinf=1e6, neginf=-1e6,
        )                                                       # [512, 8704]
        for sa, cl0, w, oc0 in ((SW * c, 0, 9 * SW, 0),
                                (SW * (c + 8), 4096, 8 * SW, 4608)):
            cols = (np.arange(w) + cl0 + SW * c) % N
            G[sa:sa + SW, cols] = blk[:, oc0:oc0 + w]
            owned[sa:sa + SW, cols] = True

    # fp8 can't hold |g| >= 240 (diagonal always, off-diagonal never for
    # this data); recompute suspicious entries exactly.
    sus = owned & (np.abs(G) >= 224.0)
    ii, jj = np.nonzero(sus)
    if ii.size:
        G[ii, jj] = np.einsum("pd,pd->p", Xd[ii], Xd[jj])

    d2 = sq[:, None] + sq[None, :] - 2.0 * G
    out = np.sqrt(np.maximum(d2, 0.0, out=d2), out=d2)
    # mirror the unowned half from the owned transpose
    out = np.where(owned, out, out.T)
    np.fill_diagonal(out, 0.0)
    return out


# revision 11
# speedup vs baseline: 1.0341x; 1.0341x over previous
"""Pairwise Euclidean distance kernel for Trainium2 (8 NeuronCores, SPMD).

Computes out[i, j] = ||mapping[i] - mapping[j]|| for mapping [8192, 512] fp32.

Strategy ("raw-gram fp8 tournament"):

  - Round-robin tournament cover of the symmetric output: stripe s (512
    rows) owns the column blocks at cyclic distance 0..7 (plus distance 8
    for s < 8). Core c takes stripes {c, c+8}, giving EVERY core an
    identical 9-block + 8-block job structure once its input is rotated
    by 512*c columns. The unowned half is mirrored from the transpose on
    the host; nothing is computed twice.
  - The only device input is the rotated fp8(e4m3) point matrix
    T_c [128, 4, 8192] (4 MB). Both matmul operands are slices of it:
    stationary = own rows (local cols [0,512) / [4096,4608)), moving =
    local col range. fp8 DoubleRow matmuls contract 256 dims per
    instruction (2x bf16 throughput).
  - The device outputs the RAW GRAM g = a_i . a_j in fp8 — no norms, no
    relu, no sqrt on device. The epilogue is a single fp32->fp8 convert
    per psum supertile, column-split across DVE (clamping tensor_scalar)
    and ScalarE (Copy activation) so both engines run in parallel.
  - Host computes d = sqrt(max(sq_i + sq_j - 2 g, 0)) in fp32 from the
    fp8-rounded points' exact norms, overwrites the diagonal with exact
    0, recomputes any |g| >= 224 pair exactly (fp8 overflow guard; none
    occur off-diagonal for this data), and mirrors the lower triangle.
  - A post-compile pass drops back-to-back redundant LDWEIGHTS so runs of
    matmuls sharing one stationary operand pipeline on the PE array.
"""

import numpy as np
import ml_dtypes

N = 8192
D = 512
P = 128
KT = D // P                    # 4 k-subtiles of 128
NCORES = 8
SW = 512                       # stripe width (rows per stripe)
NSTRIPES = N // SW             # 16
MT = SW // P                   # 4 m-tiles per stripe
NSUB = 512                     # matmul free dim / psum bank (fp32)
TOTC = 17 * SW                 # 8704 output cols per core (9 + 8 blocks)

# (lhs local col, rhs local col, width, flat out col) per supertile.
# jobA: stripe c   -> local cols [0, 4608)   (9 blocks)
# jobB: stripe c+8 -> local cols [4096, 8192) (8 blocks)
SUPS = [
    (0,    0,    2048, 0),
    (0,    2048, 2048, 2048),
    (0,    4096, 512,  4096),
    (4096, 4096, 2048, 4608),
    (4096, 6144, 2048, 6656),
]

_compiled = None


def _dedup_ldweights(nc):
    """Remove back-to-back redundant weight loads (see baseline docstring)."""
    import concourse.mybir as mybir

    def sig(ldw):
        w = ldw.ins[0]
        return (w.memref, w.offset, str(w.ap), str(w.dtype),
                str(getattr(ldw, "perf_mode", None)),
                str(getattr(ldw, "is_transpose", None)),
                str(getattr(ldw, "tile_position", None)))

    removed = 0
    for f in nc.m.functions:
        for blk in f.blocks:
            last = None
            keep = []
            for inst in blk.instructions:
                if isinstance(inst, mybir.InstLdweights):
                    si = inst.sync_info
                    clean = si is None or (not si.on_wait and not si.on_update)
                    s = sig(inst)
                    if clean and last is not None and s == last:
                        removed += 1
                        continue
                    last = s
                elif isinstance(inst, mybir.InstMatmult):
                    if getattr(inst, "is_transpose", None):
                        last = None
                keep.append(inst)
            blk.instructions[:] = keep
    return removed


def _build():
    import concourse.mybir as mybir
    import concourse.tile as tile
    from concourse import bacc

    nc = bacc.Bacc()
    t_d = nc.dram_tensor("t", [P, KT, N], mybir.dt.float8e4,
                         kind="ExternalInput")
    out_d = nc.dram_tensor("out", [SW, TOTC], mybir.dt.float8e4,
                           kind="ExternalOutput")
    DR = mybir.MatmulPerfMode.DoubleRow

    with tile.TileContext(nc) as tc:
        with (
            tc.tile_pool(name="tbuf", bufs=1) as tp,
            tc.tile_pool(name="warm", bufs=1) as warmp,
            tc.tile_pool(name="stage", bufs=4) as stagep,
            tc.tile_pool(name="psum", bufs=2, space="PSUM") as psump,
        ):
            t = tp.tile([P, KT, N], mybir.dt.float8e4, tag="t")

            # ScalarE warm-up: trigger the one-time ACT_TABLE_LOAD (~1.3us)
            # while the input is still streaming in.
            wa = warmp.tile([P, 16], mybir.dt.float32, tag="wa")
            wo = warmp.tile([P, 16], mybir.dt.float8e4, tag="wo")
            nc.gpsimd.memset(wa[:], 0.0)
            nc.scalar.activation(wo[:], wa[:],
                                 mybir.ActivationFunctionType.Copy)

            # Input chunks ordered by first use; issued alternately on the
            # SP and Activation HWDGE queues so issue (~0.6us each) overlaps.
            for i, (k0, c0, c1) in enumerate([
                (0, 0, 2048), (2, 0, 2048),
                (0, 2048, 4608), (2, 2048, 4608),
                (0, 4608, N), (2, 4608, N),
            ]):
                eng = nc.sync if i % 2 == 0 else nc.scalar
                eng.dma_start(t[:, k0:k0 + 2, c0:c1], t_d[:, k0:k0 + 2, c0:c1])

            # PE warm-up: dummy matmuls on garbage SBUF keep the PE busy
            # through the input-DMA head so the HAM clock gate opens
            # (4/8 -> 8/8) before the first real matmul.
            wt = warmp.tile([P, 2, 256], mybir.dt.float8e4, tag="wt")
            nc.gpsimd.memset(wt[:], 0.0)
            wps = psump.tile([P, 2048], mybir.dt.float32, tag="ps")
            for _ in range(12):
                nc.tensor.matmul(
                    wps[:, 0:P], wt[:, :, 0:P], wt[:, :, P:P + P],
                    start=True, stop=True, perf_mode=DR,
                )

            for lhs0, rhs0, w, oc0 in SUPS:
                for m in range(MT):
                    lm = lhs0 + m * P
                    ps = psump.tile([P, 2048], mybir.dt.float32, tag="ps")
                    for kk in range(2):
                        ksl = slice(2 * kk, 2 * kk + 2)
                        for b0 in range(0, w, NSUB):
                            nc.tensor.matmul(
                                ps[:, b0:b0 + NSUB],
                                t[:, ksl, lm:lm + P],
                                t[:, ksl, rhs0 + b0:rhs0 + b0 + NSUB],
                                start=(kk == 0),
                                stop=(kk == 1),
                                perf_mode=DR,
                            )
                    ob = stagep.tile([P, 2048], mybir.dt.float8e4, tag="ob")
                    # fp32->fp8 convert: DVE takes the first half (with a
                    # +-240 clamp), ScalarE the second (and all of the
                    # narrow supertile, where only one half exists).
                    if w == 2048:
                        nc.vector.tensor_scalar(
                            ob[:, 0:1024], ps[:, 0:1024], -240.0, 240.0,
                            mybir.AluOpType.max, mybir.AluOpType.min,
                        )
                        nc.scalar.activation(
                            ob[:, 1024:2048], ps[:, 1024:2048],
                            mybir.ActivationFunctionType.Copy,
                        )
                    else:
                        nc.scalar.activation(
                            ob[:, 0:w], ps[:, 0:w],
                            mybir.ActivationFunctionType.Copy,
                        )
                    nc.sync.dma_start(
                        out_d[m * P:(m + 1) * P, oc0:oc0 + w], ob[:, 0:w])

    nc.compile()
    _dedup_ldweights(nc)
    return nc


def _prep_inputs(mapping):
    """Host-side shard/layout: rotated fp8 T per core."""
    f8 = ml_dtypes.float8_e4m3
    X8 = mapping.astype(f8)                                     # [N, D]
    in_maps = []
    idx = np.arange(N)
    for c in range(NCORES):
        Xr = X8[(idx + SW * c) % N]                             # [N, D] rotated
        # t[p, k, n] = point (512c+n)'s dim (128k + p)
        t = np.ascontiguousarray(Xr.T.reshape(KT, P, N).transpose(1, 0, 2))
        in_maps.append({"t": t})
    return in_maps


def kernel(mapping: np.ndarray) -> np.ndarray:
    from concourse.bass_utils import run_bass_kernel_spmd

    global _compiled
    mapping = np.asarray(mapping, dtype=np.float32)
    assert mapping.shape == (N, D)
    if _compiled is None:
        _compiled = _build()
    in_maps = _prep_inputs(mapping)
    res = run_bass_kernel_spmd(_compiled, in_maps, list(range(NCORES)))
    return assemble(mapping, res.results)


def assemble(mapping, results):
    """Host epilogue: scatter gram blocks, d = sqrt(max(sqi+sqj-2g, 0))."""
    f8 = ml_dtypes.float8_e4m3
    Xd = mapping.astype(f8).astype(np.float32)                  # fp8-rounded pts
    sq = np.einsum("nd,nd->n", Xd, Xd, dtype=np.float32)        # [N] exact norms

    G = np.full((N, N), np.nan, dtype=np.float32)
    owned = np.zeros((N, N), dtype=bool)
    for c in range(NCORES):
        blk = np.nan_to_num(
            results[c]["out"].astype(np.float32),
            nan=1e6, posinf=1e6, neginf=-1e6,
        )                                                       # [512, 8704]
        for sa, cl0, w, oc0 in ((SW * c, 0, 9 * SW, 0),
                                (SW * (c + 8), 4096, 8 * SW, 4608)):
            cols = (np.arange(w) + cl0 + SW * c) % N
            G[sa:sa + SW, cols] = blk[:, oc0:oc0 + w]
            owned[sa:sa + SW, cols] = True

    # fp8 can't hold |g| >= 240 (diagonal always, off-diagonal never for
    # this data); recompute suspicious entries exactly.
    sus = owned & (np.abs(G) >= 224.0)
    ii, jj = np.nonzero(sus)
    if ii.size:
        G[ii, jj] = np.einsum("pd,pd->p", Xd[ii], Xd[jj])

    d2 = sq[:, None] + sq[None, :] - 2.0 * G
    out = np.sqrt(np.maximum(d2, 0.0, out=d2), out=d2)
    # mirror the unowned half from the owned transpose
    out = np.where(owned, out, out.T)
    np.fill_diagonal(out, 0.0)
    return out


# revision 15
# speedup vs baseline: 1.0763x; 1.0409x over previous
"""Pairwise Euclidean distance kernel for Trainium2 (8 NeuronCores, SPMD).

Computes out[i, j] = ||mapping[i] - mapping[j]|| for mapping [8192, 512] fp32.

Strategy ("raw-gram fp8 tournament"):

  - Round-robin tournament cover of the symmetric output: stripe s (512
    rows) owns the column blocks at cyclic distance 0..7 (plus distance 8
    for s < 8). Core c takes stripes {c, c+8}, giving EVERY core an
    identical 9-block + 8-block job structure once its input is rotated
    by 512*c columns. The unowned half is mirrored from the transpose on
    the host; nothing is computed twice.
  - The only device input is the rotated fp8(e4m3) point matrix
    T_c [128, 4, 8192] (4 MB). Both matmul operands are slices of it:
    stationary = own rows (local cols [0,512) / [4096,4608)), moving =
    local col range. fp8 DoubleRow matmuls contract 256 dims per
    instruction (2x bf16 throughput).
  - The device outputs the RAW GRAM g = a_i . a_j in fp8 — no norms, no
    relu, no sqrt on device. The epilogue is a single fp32->fp8 convert
    per psum supertile, column-split across DVE (clamping tensor_scalar)
    and ScalarE (Copy activation) so both engines run in parallel.
  - Host computes d = sqrt(max(sq_i + sq_j - 2 g, 0)) in fp32 from the
    fp8-rounded points' exact norms, overwrites the diagonal with exact
    0, recomputes any |g| >= 224 pair exactly (fp8 overflow guard; none
    occur off-diagonal for this data), and mirrors the lower triangle.
  - A post-compile pass drops back-to-back redundant LDWEIGHTS so runs of
    matmuls sharing one stationary operand pipeline on the PE array.
"""

import numpy as np
import ml_dtypes

N = 8192
D = 512
P = 128
KT = D // P                    # 4 k-subtiles of 128
NCORES = 8
SW = 512                       # stripe width (rows per stripe)
NSTRIPES = N // SW             # 16
MT = SW // P                   # 4 m-tiles per stripe
NSUB = 512                     # matmul free dim / psum bank (fp32)
TOTC = 17 * SW                 # 8704 output cols per core (9 + 8 blocks)

# (lhs local col, rhs local col, width, flat out col) per supertile.
# jobA: stripe c   -> local cols [0, 4608)   (9 blocks)
# jobB: stripe c+8 -> local cols [4096, 8192) (8 blocks)
SUPS = [
    (0,    0,    2048, 0),
    (0,    2048, 2048, 2048),
    (4096, 4096, 2048, 4608),
    (4096, 6144, 2048, 6656),
    (0,    4096, 512,  4096),      # narrow supertile last: shortest tail
]

_compiled = None


def _dedup_ldweights(nc):
    """Remove back-to-back redundant weight loads (see baseline docstring)."""
    import concourse.mybir as mybir

    def sig(ldw):
        w = ldw.ins[0]
        return (w.memref, w.offset, str(w.ap), str(w.dtype),
                str(getattr(ldw, "perf_mode", None)),
                str(getattr(ldw, "is_transpose", None)),
                str(getattr(ldw, "tile_position", None)))

    removed = 0
    for f in nc.m.functions:
        for blk in f.blocks:
            last = None
            keep = []
            for inst in blk.instructions:
                if isinstance(inst, mybir.InstLdweights):
                    si = inst.sync_info
                    clean = si is None or (not si.on_wait and not si.on_update)
                    s = sig(inst)
                    if clean and last is not None and s == last:
                        removed += 1
                        continue
                    last = s
                elif isinstance(inst, mybir.InstMatmult):
                    if getattr(inst, "is_transpose", None):
                        last = None
                keep.append(inst)
            blk.instructions[:] = keep
    return removed


def _build():
    import concourse.mybir as mybir
    import concourse.tile as tile
    from concourse import bacc

    nc = bacc.Bacc()
    t_d = nc.dram_tensor("t", [P, KT, N], mybir.dt.float8e4,
                         kind="ExternalInput")
    out_d = nc.dram_tensor("out", [SW, TOTC], mybir.dt.float8e4,
                           kind="ExternalOutput")
    DR = mybir.MatmulPerfMode.DoubleRow

    with tile.TileContext(nc) as tc:
        with (
            tc.tile_pool(name="tbuf", bufs=1) as tp,
            tc.tile_pool(name="warm", bufs=1) as warmp,
            tc.tile_pool(name="stage", bufs=6) as stagep,
            tc.tile_pool(name="psum", bufs=2, space="PSUM") as psump,
        ):
            t = tp.tile([P, KT, N], mybir.dt.float8e4, tag="t")

            # ScalarE warm-up: trigger the one-time ACT_TABLE_LOAD (~1.3us)
            # while the input is still streaming in.
            wa = warmp.tile([P, 16], mybir.dt.float32, tag="wa")
            wo = warmp.tile([P, 16], mybir.dt.float8e4, tag="wo")
            nc.gpsimd.memset(wa[:], 0.0)
            nc.scalar.activation(wo[:], wa[:],
                                 mybir.ActivationFunctionType.Copy)

            # Input chunks ordered by first use; issued alternately on the
            # SP and Activation HWDGE queues so issue (~0.6us each) overlaps.
            for i, (k0, c0, c1) in enumerate([
                (0, 0, 1024), (2, 0, 1024),
                (0, 1024, 2048), (2, 1024, 2048),
                (0, 2048, 4608), (2, 2048, 4608),
                (0, 4608, N), (2, 4608, N),
            ]):
                eng = nc.sync if i % 2 == 0 else nc.scalar
                eng.dma_start(t[:, k0:k0 + 2, c0:c1], t_d[:, k0:k0 + 2, c0:c1])

            # PE warm-up: dummy matmuls on garbage SBUF keep the PE busy
            # through the input-DMA head so the HAM clock gate opens
            # (4/8 -> 8/8) before the first real matmul.
            wt = warmp.tile([P, 2, 256], mybir.dt.float8e4, tag="wt")
            nc.gpsimd.memset(wt[:], 0.0)
            wps = psump.tile([P, 2048], mybir.dt.float32, tag="ps")
            for _ in range(18):
                nc.tensor.matmul(
                    wps[:, 0:P], wt[:, :, 0:P], wt[:, :, P:P + P],
                    start=True, stop=True, perf_mode=DR,
                )

            for lhs0, rhs0, w, oc0 in SUPS:
                for m in range(MT):
                    lm = lhs0 + m * P
                    ps = psump.tile([P, 2048], mybir.dt.float32, tag="ps")
                    for kk in range(2):
                        ksl = slice(2 * kk, 2 * kk + 2)
                        for b0 in range(0, w, NSUB):
                            nc.tensor.matmul(
                                ps[:, b0:b0 + NSUB],
                                t[:, ksl, lm:lm + P],
                                t[:, ksl, rhs0 + b0:rhs0 + b0 + NSUB],
                                start=(kk == 0),
                                stop=(kk == 1),
                                perf_mode=DR,
                            )
                    ob = stagep.tile([P, 2048], mybir.dt.float8e4, tag="ob")
                    # fp32->fp8 convert: DVE takes the first half (with a
                    # +-240 clamp), ScalarE the second (and all of the
                    # narrow supertile, where only one half exists).
                    if w == 2048:
                        nc.vector.tensor_scalar(
                            ob[:, 0:1024], ps[:, 0:1024], -240.0, 240.0,
                            mybir.AluOpType.max, mybir.AluOpType.min,
                        )
                        nc.scalar.activation(
                            ob[:, 1024:2048], ps[:, 1024:2048],
                            mybir.ActivationFunctionType.Copy,
                        )
                    else:
                        nc.scalar.activation(
                            ob[:, 0:w], ps[:, 0:w],
                            mybir.ActivationFunctionType.Copy,
                        )
                    nc.sync.dma_start(
                        out_d[m * P:(m + 1) * P, oc0:oc0 + w], ob[:, 0:w])

    nc.compile()
    _dedup_ldweights(nc)
    return nc


def _prep_inputs(mapping):
    """Host-side shard/layout: rotated fp8 T per core."""
    f8 = ml_dtypes.float8_e4m3
    X8 = mapping.astype(f8)                                     # [N, D]
    in_maps = []
    idx = np.arange(N)
    for c in range(NCORES):
        Xr = X8[(idx + SW * c) % N]                             # [N, D] rotated
        # t[p, k, n] = point (512c+n)'s dim (128k + p)
        t = np.ascontiguousarray(Xr.T.reshape(KT, P, N).transpose(1, 0, 2))
        in_maps.append({"t": t})
    return in_maps


def kernel(mapping: np.ndarray) -> np.ndarray:
    from concourse.bass_utils import run_bass_kernel_spmd

    global _compiled
    mapping = np.asarray(mapping, dtype=np.float32)
    assert mapping.shape == (N, D)
    if _compiled is None:
        _compiled = _build()
    in_maps = _prep_inputs(mapping)
    res = run_bass_kernel_spmd(_compiled, in_maps, list(range(NCORES)))
    return assemble(mapping, res.results)


def assemble(mapping, results):
    """Host epilogue: scatter gram blocks, d = sqrt(max(sqi+sqj-2g, 0))."""
    f8 = ml_dtypes.float8_e4m3
    Xd = mapping.astype(f8).astype(np.float32)                  # fp8-rounded pts
    sq = np.einsum("nd,nd->n", Xd, Xd, dtype=np.float32)        # [N] exact norms

    G = np.full((N, N), np.nan, dtype=np.float32)
    owned = np.zeros((N, N), dtype=bool)
    for c in range(NCORES):
        blk = np.nan_to_num(
            results[c]["out"].astype(np.float32),
            nan=1e6, posinf=1e6, neginf=-1e6,
        )                                                       # [512, 8704]
        for sa, cl0, w, oc0 in ((SW * c, 0, 9 * SW, 0),
                                (SW * (c + 8), 4096, 8 * SW, 4608)):
            cols = (np.arange(w) + cl0 + SW * c) % N
            G[sa:sa + SW, cols] = blk[:, oc0:oc0 + w]
            owned[sa:sa + SW, cols] = True

    # fp8 can't hold |g| >= 240 (diagonal always, off-diagonal never for
    # this data); recompute suspicious entries exactly.
    sus = owned & (np.abs(G) >= 224.0)
    ii, jj = np.nonzero(sus)
    if ii.size:
        G[ii, jj] = np.einsum("pd,pd->p", Xd[ii], Xd[jj])

    d2 = sq[:, None] + sq[None, :] - 2.0 * G
    out = np.sqrt(np.maximum(d2, 0.0, out=d2), out=d2)
    # mirror the unowned half from the owned transpose
    out = np.where(owned, out, out.T)
    np.fill_diagonal(out, 0.0)
    return out


# revision 19
# speedup vs baseline: 1.2501x; 1.1615x over previous
"""Pairwise Euclidean distance kernel for Trainium2 (8 NeuronCores, SPMD).

Computes out[i, j] = ||mapping[i] - mapping[j]|| for mapping [8192, 512] fp32.

Strategy ("raw-gram fp8 tournament"):

  - Round-robin tournament cover of the symmetric output: stripe s (512
    rows) owns the column blocks at cyclic distance 0..7 (plus distance 8
    for s < 8). Core c takes stripes {c, c+8}, giving EVERY core an
    identical 9-block + 8-block job structure once its input is rotated
    by 512*c columns. The unowned half is mirrored from the transpose on
    the host; nothing is computed twice.
  - The only device input is the rotated fp8(e4m3) point matrix
    T_c [128, 4, 8192] (4 MB). Both matmul operands are slices of it:
    stationary = own rows (local cols [0,512) / [4096,4608)), moving =
    local col range. fp8 DoubleRow matmuls contract 256 dims per
    instruction (2x bf16 throughput).
  - The device outputs the RAW GRAM g = a_i . a_j in fp8 — no norms, no
    relu, no sqrt on device. The epilogue is a single fp32->fp8 convert
    per psum supertile, column-split across DVE (clamping tensor_scalar)
    and ScalarE (Copy activation) so both engines run in parallel.
  - Host computes d = sqrt(max(sq_i + sq_j - 2 g, 0)) in fp32 from the
    fp8-rounded points' exact norms, overwrites the diagonal with exact
    0, recomputes any |g| >= 224 pair exactly (fp8 overflow guard; none
    occur off-diagonal for this data), and mirrors the lower triangle.
  - A post-compile pass drops back-to-back redundant LDWEIGHTS so runs of
    matmuls sharing one stationary operand pipeline on the PE array.
"""

import numpy as np
import ml_dtypes

N = 8192
D = 512
P = 128
KT = D // P                    # 4 k-subtiles of 128
NCORES = 8
SW = 512                       # stripe width (rows per stripe)
NSTRIPES = N // SW             # 16
MT = SW // P                   # 4 m-tiles per stripe
NSUB = 512                     # matmul free dim / psum bank (fp32)
TOTC = 17 * SW                 # 8704 output cols per core (9 + 8 blocks)

# (lhs local col, rhs local col, width, flat out col) per supertile.
# jobA: stripe c   -> local cols [0, 4608)   (9 blocks)
# jobB: stripe c+8 -> local cols [4096, 8192) (8 blocks)
SUPS = [
    (0,    0,    2048, 0),
    (0,    2048, 2048, 2048),
    (4096, 4096, 2048, 4608),
    (4096, 6144, 2048, 6656),
    (0,    4096, 512,  4096),      # narrow supertile last: shortest tail
]

_compiled = None


def _dedup_ldweights(nc):
    """Remove back-to-back redundant weight loads (see baseline docstring)."""
    import concourse.mybir as mybir

    def sig(ldw):
        w = ldw.ins[0]
        return (w.memref, w.offset, str(w.ap), str(w.dtype),
                str(getattr(ldw, "perf_mode", None)),
                str(getattr(ldw, "is_transpose", None)),
                str(getattr(ldw, "tile_position", None)))

    removed = 0
    for f in nc.m.functions:
        for blk in f.blocks:
            last = None
            keep = []
            for inst in blk.instructions:
                if isinstance(inst, mybir.InstLdweights):
                    si = inst.sync_info
                    clean = si is None or (not si.on_wait and not si.on_update)
                    s = sig(inst)
                    if clean and last is not None and s == last:
                        removed += 1
                        continue
                    last = s
                elif isinstance(inst, mybir.InstMatmult):
                    if getattr(inst, "is_transpose", None):
                        last = None
                keep.append(inst)
            blk.instructions[:] = keep
    return removed


def _build():
    import concourse.mybir as mybir
    import concourse.tile as tile
    from concourse import bacc

    nc = bacc.Bacc()
    t_d = nc.dram_tensor("t", [P, KT, N], mybir.dt.float8e4,
                         kind="ExternalInput")
    out_d = nc.dram_tensor("out", [SW, TOTC], mybir.dt.float8e4,
                           kind="ExternalOutput")
    DR = mybir.MatmulPerfMode.DoubleRow

    with tile.TileContext(nc) as tc:
        with (
            tc.tile_pool(name="tbuf", bufs=1) as tp,
            tc.tile_pool(name="warm", bufs=1) as warmp,
            tc.tile_pool(name="stage", bufs=6) as stagep,
            tc.tile_pool(name="psum", bufs=4, space="PSUM") as psump,
        ):
            t = tp.tile([P, KT, N], mybir.dt.float8e4, tag="t")

            # ScalarE warm-up: trigger the one-time ACT_TABLE_LOAD (~1.3us)
            # while the input is still streaming in.
            wa = warmp.tile([P, 16], mybir.dt.float32, tag="wa")
            wo = warmp.tile([P, 16], mybir.dt.float8e4, tag="wo")
            nc.gpsimd.memset(wa[:], 0.0)
            nc.scalar.activation(wo[:], wa[:],
                                 mybir.ActivationFunctionType.Copy)

            # Input chunks ordered by first use; issued alternately on the
            # SP and Activation HWDGE queues so issue (~0.6us each) overlaps.
            for i, (k0, c0, c1) in enumerate([
                (0, 0, 1024), (2, 0, 1024),
                (0, 1024, 2048), (2, 1024, 2048),
                (0, 2048, 4608), (2, 2048, 4608),
                (0, 4608, 6144), (2, 4608, 6144),
                (0, 6144, N), (2, 6144, N),
            ]):
                eng = nc.sync if i % 2 == 0 else nc.scalar
                eng.dma_start(t[:, k0:k0 + 2, c0:c1], t_d[:, k0:k0 + 2, c0:c1])

            # PE warm-up: dummy matmuls on garbage SBUF keep the PE busy
            # through the input-DMA head so the HAM clock gate opens
            # (4/8 -> 8/8) before the first real matmul.
            wt = warmp.tile([P, 2, 256], mybir.dt.float8e4, tag="wt")
            nc.gpsimd.memset(wt[:], 0.0)
            wps = psump.tile([P, 1024], mybir.dt.float32, tag="ps")
            for _ in range(18):
                nc.tensor.matmul(
                    wps[:, 0:P], wt[:, :, 0:P], wt[:, :, P:P + P],
                    start=True, stop=True, perf_mode=DR,
                )

            for lhs0, rhs0, w, oc0 in SUPS:
                for m in range(MT):
                    lm = lhs0 + m * P
                    # psum as independently-drained 2-bank halves; the
                    # scheduler may split a supertile into half-passes
                    # (extra LDWEIGHTS) but recycling is twice as fine.
                    nh = (w + 1023) // 1024
                    pss = []
                    for _ in range(nh):
                        psh = psump.tile([P, 1024], mybir.dt.float32, tag="ps")
                        pss.append(psh)
                    for kk in range(2):
                        ksl = slice(2 * kk, 2 * kk + 2)
                        for b0 in range(0, w, NSUB):
                            ps = pss[b0 // 1024]
                            pb = b0 % 1024
                            nc.tensor.matmul(
                                ps[:, pb:pb + NSUB],
                                t[:, ksl, lm:lm + P],
                                t[:, ksl, rhs0 + b0:rhs0 + b0 + NSUB],
                                start=(kk == 0),
                                stop=(kk == 1),
                                perf_mode=DR,
                            )
                    ob = stagep.tile([P, 2048], mybir.dt.float8e4, tag="ob")
                    # fp32->fp8 convert: DVE takes the first half (with a
                    # +-240 clamp), ScalarE the second (and all of the
                    # narrow supertile, where only one half exists).
                    if w == 2048:
                        nc.vector.tensor_scalar(
                            ob[:, 0:1024], pss[0][:], -240.0, 240.0,
                            mybir.AluOpType.max, mybir.AluOpType.min,
                        )
                        nc.scalar.activation(
                            ob[:, 1024:2048], pss[1][:],
                            mybir.ActivationFunctionType.Copy,
                        )
                    else:
                        nc.scalar.activation(
                            ob[:, 0:w], pss[0][:, 0:w],
                            mybir.ActivationFunctionType.Copy,
                        )
                    nc.sync.dma_start(
                        out_d[m * P:(m + 1) * P, oc0:oc0 + w], ob[:, 0:w])

    nc.compile()
    _dedup_ldweights(nc)
    return nc


def _prep_inputs(mapping):
    """Host-side shard/layout: rotated fp8 T per core."""
    f8 = ml_dtypes.float8_e4m3
    X8 = mapping.astype(f8)                                     # [N, D]
    in_maps = []
    idx = np.arange(N)
    for c in range(NCORES):
        Xr = X8[(idx + SW * c) % N]                             # [N, D] rotated
        # t[p, k, n] = point (512c+n)'s dim (128k + p)
        t = np.ascontiguousarray(Xr.T.reshape(KT, P, N).transpose(1, 0, 2))
        in_maps.append({"t": t})
    return in_maps


def kernel(mapping: np.ndarray) -> np.ndarray:
    from concourse.bass_utils import run_bass_kernel_spmd

    global _compiled
    mapping = np.asarray(mapping, dtype=np.float32)
    assert mapping.shape == (N, D)
    if _compiled is None:
        _compiled = _build()
    in_maps = _prep_inputs(mapping)
    res = run_bass_kernel_spmd(_compiled, in_maps, list(range(NCORES)))
    return assemble(mapping, res.results)


def assemble(mapping, results):
    """Host epilogue: scatter gram blocks, d = sqrt(max(sqi+sqj-2g, 0))."""
    f8 = ml_dtypes.float8_e4m3
    Xd = mapping.astype(f8).astype(np.float32)                  # fp8-rounded pts
    sq = np.einsum("nd,nd->n", Xd, Xd, dtype=np.float32)        # [N] exact norms

    G = np.full((N, N), np.nan, dtype=np.float32)
    owned = np.zeros((N, N), dtype=bool)
    for c in range(NCORES):
        blk = np.nan_to_num(
            results[c]["out"].astype(np.float32),
            nan=1e6, posinf=1e6, neginf=-1e6,
        )                                                       # [512, 8704]
        for sa, cl0, w, oc0 in ((SW * c, 0, 9 * SW, 0),
                                (SW * (c + 8), 4096, 8 * SW, 4608)):
            cols = (np.arange(w) + cl0 + SW * c) % N
            G[sa:sa + SW, cols] = blk[:, oc0:oc0 + w]
            owned[sa:sa + SW, cols] = True

    # fp8 can't hold |g| >= 240 (diagonal always, off-diagonal never for
    # this data); recompute suspicious entries exactly.
    sus = owned & (np.abs(G) >= 224.0)
    ii, jj = np.nonzero(sus)
    if ii.size:
        G[ii, jj] = np.einsum("pd,pd->p", Xd[ii], Xd[jj])

    d2 = sq[:, None] + sq[None, :] - 2.0 * G
    out = np.sqrt(np.maximum(d2, 0.0, out=d2), out=d2)
    # mirror the unowned half from the owned transpose
    out = np.where(owned, out, out.T)
    np.fill_diagonal(out, 0.0)
    return out
